# revision 32
# baseline (speedup 1.0000x reference)
"""Trainium2 Bass kernel for nn_Attention (b=8, n=1024, dim=768, heads=12).

Sharding: data-parallel over batch — 8 batch elements -> 8 NeuronCores.
Each core runs full attention for one [1024, 768] slice; weights replicated.

Design (v1, software-pipelined):
  - exp is ACT-engine-only (no DVE/Pool exp on TRN2) and totals ~82us of the
    ~150us of PE matmul work, so attention is emitted as an ACT/PE pipeline
    and the q/k/v projections are interleaved INTO the attention rounds so PE
    fills its exp-wait slack with projection matmuls.
  - i-dim halved (512) in attention so PSUM fits: sts [128,2,512] (2 banks,
    x2 bufs) + AV accumulators [128,2,512] (2 banks) + proj fill (2 banks).
  - one exp instruction per round covers both heads of a pair (halves ACT
    instruction overhead vs per-head exps).
  - softmax normalize reads PSUM directly (no staging copy): reciprocal(DVE)
    + partition_broadcast(Pool) + multiply(DVE).
  - DMA split across both HWDGE queues: SP carries x strips + qk weight
    tiles + output stores; ACT queue carries the big wv/wo/bias loads.
"""

import numpy as np
from contextlib import ExitStack

import concourse.bacc as bacc
import concourse.mybir as mybir
import concourse.tile as tile
from concourse.bass import ds, ts
from concourse.bass_utils import run_bass_kernel_spmd
from concourse.masks import make_identity

P = 128
N_CORES = 8
N_TOK = 1024
DIM = 768
H = 12
HD = 64
SCALE = 1.0 / (DIM ** 0.5)
F32 = mybir.dt.float32
F32R = mybir.dt.float32r
BF16 = mybir.dt.bfloat16
FP8 = mybir.dt.float8e4
DR = mybir.MatmulPerfMode.DoubleRow
QK_FP8 = False  # False: bf16 q/k stores, plain matmul (more accuracy margin)
EXP = mybir.ActivationFunctionType.Exp

C_T = DIM // P          # 6  c-tiles
N_T = N_TOK // P        # 8  token tiles
IH = 512                # attention i-chunk (half of n per (pr, ihalf) pass)


def _emit_body(nc, tc, ctx, pools, dram):
    x_d, wqkv_d, wout_d, bout_d, out_d = dram
    const, persist, xpool, wpool, expool, npool, outpool, psS, psO, psP = pools

    wqkv_r = wqkv_d.rearrange("(o p) f -> p o f", p=P)
    wout_r = wout_d.rearrange("(o p) f -> p o f", p=P)

    # ---- constants ----
    identity = const.tile([P, P], F32, tag="ident")
    make_identity(nc, identity[:])

    # ---- persistent tensors ----
    # q/k stores are fp8e4 so QK^T runs in DoubleRow mode (0.5 cycles/row).
    # DoubleRow contracts two 64-row subtiles per instruction; the second
    # subtile of kT8 is zeroed, so its paired qT8 data is multiplied by 0.
    # qT8 carries a 512-col zeroed pad so the i-half-1 window's second
    # subtile reads in-bounds, finite data.
    xT = persist.tile([P, C_T, N_TOK], F32R, tag="xT")
    if QK_FP8:
        qT8 = persist.tile([P, 6, N_TOK + IH], FP8, tag="qT8")
        kT8 = persist.tile([P, 6, N_T, 2, P], FP8, tag="kT8")  # [pair, jt, sub, j]
    else:
        qkT = persist.tile([P, 6, 2, N_TOK], BF16, tag="qkT")  # [pair, q/k, n]
    vplus = persist.tile([P, N_T, H, HD + 1], F32R, tag="vplus")
    wv_sb = persist.tile([P, C_T, DIM], F32R, tag="wv")
    wo_sb = persist.tile([P, C_T, DIM], F32R, tag="wo")
    attnT = persist.tile([P, C_T, N_TOK], F32R, tag="attnT")

    nc.vector.memset(vplus[:, :, :, ds(HD, 1)].bitcast(F32), 1.0)
    # zero fills on the idle Pool engine: kT8's second subtiles, and qT8's
    # not-yet-written/pad columns that early rounds read as x0 garbage.
    # pair 0's slices go first so round (0,0,0) isn't gated on the bulk.
    if QK_FP8:
        nc.gpsimd.memset(kT8[:, 0, :, 1, :], 0.0)
        nc.gpsimd.memset(qT8[:, 0, ds(IH, N_TOK)], 0.0)
        nc.gpsimd.memset(kT8[:, ds(1, 5), :, 1, :], 0.0)
        nc.gpsimd.memset(qT8[:, ds(1, 5), ds(IH, N_TOK)], 0.0)

    # ---- DMA emission: one ordered queue, criticality order ----
    # xs0-3 -> pair0 weights -> wv (V units start ~round 0) -> xs4-7 ->
    # pair1 weights -> bias -> wo (needed only at the tail).
    xs_tiles = []
    wqk_tiles = {}

    def load_wqk(fi):
        wt = wpool.tile([P, C_T, P], F32R, tag="wqk", name=f"wqk_{fi}")
        nc.sync.dma_start(wt[:], wqkv_r[:, :, ds(fi * P, P)].bitcast(F32R))
        wqk_tiles[fi] = wt

    for it in range(N_T):
        xs = xpool.tile([P, DIM], F32, tag="xs", name=f"xs_{it}")
        if it < 4:
            nc.sync.dma_start(xs[:], x_d[ts(it, P), :])
        xs_tiles.append(xs)
    load_wqk(0)
    load_wqk(6)
    nc.sync.dma_start(wv_sb[:], wqkv_r[:, :, ds(2 * DIM, DIM)].bitcast(F32R))
    for it in range(4, N_T):
        nc.sync.dma_start(xs_tiles[it][:], x_d[ts(it, P), :])
    load_wqk(1)
    load_wqk(7)
    b_row = const.tile([1, DIM], F32, tag="brow")
    nc.sync.dma_start(b_row[:], bout_d[None, :])
    bias_bc = const.tile([P, DIM], F32, tag="bias")
    nc.gpsimd.partition_broadcast(bias_bc[:], b_row[:])
    nc.sync.dma_start(wo_sb[:], wout_r[:].bitcast(F32R))

    # ---- x strip transpose (6 PE transposes + 3 merged DVE copies) ----
    def transpose_strip(it):
        xs = xs_tiles[it]
        for c in range(0, C_T, 2):
            pst = psS.tile([P, 2, IH], F32, tag="ps", name=f"t_{it}_{c}")
            nc.tensor.transpose(pst[:, 0, 0:P], xs[:, ts(c, P)], identity[:])
            nc.tensor.transpose(pst[:, 1, 0:P], xs[:, ts(c + 1, P)], identity[:])
            nc.vector.tensor_copy(xT[:, c:c + 2, ts(it, P)], pst[:, :, 0:P])

    # ---- projection units ----
    def qk_half(pair, qk, nh, pool):
        """half f-tile (fi = pair + 6*qk, n-half nh) -> qT8/kT8 (fp8 cast)."""
        fi = pair + 6 * qk
        wt = wqk_tiles[fi]
        ps = pool.tile([P, IH], F32, tag="ps" if pool is psS else "pp",
                       name=f"qk_{fi}_{nh}")
        for k in range(C_T):
            nc.tensor.matmul(
                ps[:], wt[:, k], xT[:, k, ds(nh * IH, IH)],
                start=(k == 0), stop=(k == C_T - 1))
        if not QK_FP8:
            nc.vector.tensor_copy(qkT[:, pair, qk, ds(nh * IH, IH)], ps[:])
        elif qk == 0:
            nc.vector.tensor_copy(qT8[:, pair, ds(nh * IH, IH)], ps[:])
        else:
            nc.vector.tensor_copy(
                kT8[:, pair, ds(nh * 4, 4), 0, :],
                ps[:].rearrange("p (j m) -> p j m", m=P))

    def v_half(jt, dh, pool):
        """V projection for d-chunk dh (0: d 0-511 / heads 0-7, 1: d 512-767
        / heads 8-11)."""
        off, w = (0, 512) if dh == 0 else (512, 256)
        ps = pool.tile([P, 512], F32, tag="ps" if pool is psS else "pp",
                       name=f"v_{jt}_{dh}")
        for k in range(C_T):
            nc.tensor.matmul(
                ps[:, ds(0, w)], xT[:, k, ts(jt, P)], wv_sb[:, k, ds(off, w)],
                start=(k == 0), stop=(k == C_T - 1))
        nc.vector.tensor_copy(
            vplus[:, jt, ds(off // HD, w // HD), ds(0, HD)],
            ps[:, :w].rearrange("p (h d) -> p h d", d=HD))

    # ---- pre-attention critical path: strips 0-3, pair-0 nh0 halves ----
    for it in range(4):
        transpose_strip(it)
    qk_half(0, 0, 0, psP)       # qT pair0, i-half 0
    qk_half(0, 1, 0, psS)       # kT pair0, j-tiles 0-3

    # ---- interleave schedule: everything else fills attention rounds ----
    # (pr, ihalf, jt) -> list of emit-callbacks run between QK(r+1) and AV(r).
    units = {}

    def at(pr, ihalf, jt, fn):
        units.setdefault((pr, ihalf, jt), []).append(fn)

    # segment 0: strips 4-7, V units just-in-time, pair-0 remaining halves
    at(0, 0, 0, lambda: transpose_strip(4))
    at(0, 0, 0, lambda: transpose_strip(5))
    at(0, 0, 0, lambda: v_half(0, 0, psP))
    at(0, 0, 0, lambda: v_half(0, 1, psP))
    at(0, 0, 1, lambda: transpose_strip(6))
    at(0, 0, 1, lambda: transpose_strip(7))
    at(0, 0, 1, lambda: v_half(1, 0, psP))
    at(0, 0, 1, lambda: v_half(1, 1, psP))
    at(0, 0, 2, lambda: qk_half(0, 1, 1, psP))      # kT pair0 j-tiles 4-7
    for j in range(2, N_T):     # V_j in round j, always ahead of AV(j)
        at(0, 0, j, lambda j=j: v_half(j, 0, psP))
        at(0, 0, j, lambda j=j: v_half(j, 1, psP))
    at(0, 0, 6, lambda: qk_half(0, 0, 1, psP))      # qT pair0, i-half 1
    # pair p>=1: all four halves inside segment 2p-1
    for p in range(1, 6):
        pr_, ih_ = divmod(2 * p - 1, 2)
        at(pr_, ih_, 0, lambda p=p: qk_half(p, 0, 0, psP))
        at(pr_, ih_, 2, lambda p=p: qk_half(p, 1, 0, psP))
        at(pr_, ih_, 4, lambda p=p: qk_half(p, 1, 1, psP))
        at(pr_, ih_, 6, lambda p=p: qk_half(p, 0, 1, psP))
        if p + 1 <= 5:          # prefetch pair p+1's weight tiles
            at(pr_, ih_, 0, lambda f=p + 1: load_wqk(f))
            at(pr_, ih_, 0, lambda f=p + 7: load_wqk(f))

    # out-projection k=0..4 partials run inside the (ACT-bound) final
    # segments; results stage into xT's SBUF, which is dead by then. Only
    # the k=5 matmul + add + store remain in the tail. Writes go through an
    # f32r-typed view (xT's location feeds f32r matmuls, and the verifier
    # requires f32r-rounded producers); tail reads use a plain-f32 bitcast.
    part_wr = xT.rearrange("p a b -> p (a b)")
    part_sb = xT.bitcast(F32).rearrange("p a b -> p (a b)")

    def part_unit(it):
        pp = psS.tile([P, 1024], F32, tag="ps", name=f"part_{it}")
        for k in range(5):
            for off, w in ((0, 512), (512, 256)):
                nc.tensor.matmul(
                    pp[:, ds(off, w)], attnT[:, k, ts(it, P)], wo_sb[:, k, ds(off, w)],
                    start=(k == 0), stop=(k == 4))
        nc.vector.tensor_add(part_wr[:, ds(it * DIM, DIM)], pp[:, :DIM],
                             bias_bc[:])

    for it in range(N_T):
        pr_, ih_ = divmod(10 + it // 4, 2)
        at(pr_, ih_, 2 * (it % 4), lambda it=it: part_unit(it))

    # ---- attention: flat pipeline over all (pr, ihalf, jt) rounds ----
    # Emission order per round r: exp(r), QK(r+1), proj-units, AV(r) — so the
    # PE's next QK is never queued behind an exp-gated AV, and ACT never
    # starves on the round chain.
    rounds = [(pr, ih, jt) for pr in range(6) for ih in range(2)
              for jt in range(N_T)]

    def emit_qk(pr, ihalf, jt):
        sts = psS.tile([P, 2, IH], F32, tag="ps", name=f"st_{pr}_{ihalf}_{jt}")
        for sub in range(2):
            b0 = HD * sub
            if QK_FP8:
                nc.tensor.matmul(
                    sts[:, sub, :],
                    kT8[b0:b0 + HD, pr, jt, :, :],
                    qT8[b0:b0 + HD, pr, ds(ihalf * IH, 2 * IH)].rearrange(
                        "p (s i) -> p s i", s=2),
                    start=True, stop=True, perf_mode=DR, tile_position=(b0, 0))
            else:
                nc.tensor.matmul(
                    sts[:, sub, :],
                    qkT[b0:b0 + HD, pr, 1, ts(jt, P)],
                    qkT[b0:b0 + HD, pr, 0, ds(ihalf * IH, IH)],
                    start=True, stop=True, tile_position=(b0, 0))
        return sts

    o_ps = None
    sts = emit_qk(*rounds[0])
    for i, (pr, ihalf, jt) in enumerate(rounds):
        es = expool.tile([P, 2, IH], F32R, tag="es",
                         name=f"es_{pr}_{ihalf}_{jt}")
        nc.scalar.activation(es[:], sts[:], EXP, scale=SCALE)
        sts_next = emit_qk(*rounds[i + 1]) if i + 1 < len(rounds) else None
        for fn in units.get((pr, ihalf, jt), ()):
            fn()
        if jt == 0:
            o_ps = psO.tile([P, 2, IH], F32, tag="po", name=f"o_{pr}_{ihalf}")
        for sub in range(2):
            nc.tensor.matmul(
                o_ps[0:HD + 1, sub, :], vplus[:, jt, 2 * pr + sub],
                es[:, sub, :],
                start=(jt == 0), stop=(jt == N_T - 1))
        if jt == N_T - 1:
            last_seg = i == len(rounds) - 1
            if last_seg:
                # final segment: psO is never reused — normalize straight
                # from PSUM, off the staging-copy latency
                src = o_ps
            else:
                # one staging copy frees psO fast; normalize off SBUF after
                src = npool.tile([HD + 1, 2, IH], F32, tag="ostg", bufs=1,
                                 name=f"ostg_{pr}_{ihalf}")
                nc.vector.tensor_copy(src[:], o_ps[0:HD + 1, :, :])
            for sub in range(2):
                rec = npool.tile([1, IH], F32, tag="rec")
                nc.vector.reciprocal(rec[:], src[HD:HD + 1, sub, :])
                rb = npool.tile([HD, IH], F32, tag="rb")
                nc.gpsimd.partition_broadcast(rb[:], rec[:])
                nc.vector.tensor_mul(
                    attnT[HD * sub:HD * (sub + 1), pr, ds(ihalf * IH, IH)],
                    src[0:HD, sub, :], rb[:])
        sts = sts_next

    # ---- output projection tail: k=5 contribution + staged partial ----
    # alternate psS/psP so four tiles are in flight at the end
    for it in range(N_T):
        if it % 2 == 0:
            ps = psS.tile([P, 1024], F32, tag="ps", name=f"o4_{it}")
        else:
            ps = psP.tile([P, 512], F32, tag="pp", name=f"o4_{it}")
        for ci, (off, w) in enumerate(((0, 512), (512, 256))):
            po = ps[:, ds(off, w)] if it % 2 == 0 else (
                ps[:, ds(0, 512)] if ci == 0 else None)
            if po is None:
                # odd tiles: second chunk goes to the other psP buffer
                ps2 = psP.tile([P, 512], F32, tag="pp", name=f"o4b_{it}")
                po = ps2[:, ds(0, w)]
            nc.tensor.matmul(
                po, attnT[:, 5, ts(it, P)], wo_sb[:, 5, ds(off, w)],
                start=True, stop=True)
        if it < 4:
            os = outpool.tile([P, DIM], F32, tag="os", name=f"os_{it}")
        else:
            # x strip staging tiles are dead by now and exactly [128, DIM]:
            # reuse them as extra store buffers so the last adds never wait
            # on a completed DMA to free a slot
            os = xpool.tile([P, DIM], F32, tag="xs", name=f"os_{it}")
        if it % 2 == 0:
            nc.vector.tensor_add(os[:], ps[:, :DIM],
                                 part_sb[:, ds(it * DIM, DIM)])
        else:
            nc.vector.tensor_add(os[:, 0:512], ps[:, 0:512],
                                 part_sb[:, ds(it * DIM, 512)])
            nc.vector.tensor_add(os[:, 512:DIM], ps2[:, 0:256],
                                 part_sb[:, ds(it * DIM + 512, 256)])
        nc.sync.dma_start(out_d[ts(it, P), :], os[:])


def build_nc(reps: int = 1, timing_mode: bool = False):
    nc = bacc.Bacc("TRN2", target_bir_lowering=False, debug=False)
    if timing_mode:
        # device-resident garbage inputs: measure kernel exec, not host I/O
        x_d = nc.dram_tensor("x", [N_TOK, DIM], F32).ap()
        wqkv_d = nc.dram_tensor("w_qkv", [DIM, 3 * DIM], F32).ap()
        wout_d = nc.dram_tensor("w_out", [DIM, DIM], F32).ap()
        bout_d = nc.dram_tensor("b_out", [DIM], F32).ap()
        out_d = nc.dram_tensor("out", [N_TOK, DIM], F32).ap()
        dummy_in = nc.dram_tensor("dummy_in", [1, 1], F32, kind="ExternalInput").ap()
        tiny_out = nc.dram_tensor("tiny_out", [1, 1], F32, kind="ExternalOutput").ap()
    else:
        x_d = nc.dram_tensor("x", [N_TOK, DIM], F32, kind="ExternalInput").ap()
        wqkv_d = nc.dram_tensor("w_qkv", [DIM, 3 * DIM], F32, kind="ExternalInput").ap()
        wout_d = nc.dram_tensor("w_out", [DIM, DIM], F32, kind="ExternalInput").ap()
        bout_d = nc.dram_tensor("b_out", [DIM], F32, kind="ExternalInput").ap()
        out_d = nc.dram_tensor("out", [N_TOK, DIM], F32, kind="ExternalOutput").ap()
    dram = (x_d, wqkv_d, wout_d, bout_d, out_d)

    with ExitStack() as ctx:
        tc = ctx.enter_context(tile.TileContext(nc))
        const = ctx.enter_context(tc.tile_pool(name="const", bufs=1))
        persist = ctx.enter_context(tc.tile_pool(name="persist", bufs=1))
        xpool = ctx.enter_context(tc.tile_pool(name="xpool", bufs=3))
        wpool = ctx.enter_context(tc.tile_pool(name="wpool", bufs=3))
        expool = ctx.enter_context(tc.tile_pool(name="expool", bufs=4))
        npool = ctx.enter_context(tc.tile_pool(name="npool", bufs=2))
        outpool = ctx.enter_context(tc.tile_pool(name="outpool", bufs=2))
        psS = ctx.enter_context(tc.tile_pool(name="psS", bufs=2, space="PSUM"))
        psO = ctx.enter_context(tc.tile_pool(name="psO", bufs=1, space="PSUM"))
        psP = ctx.enter_context(tc.tile_pool(name="psP", bufs=2, space="PSUM"))
        pools = (const, persist, xpool, wpool, expool, npool, outpool, psS, psO, psP)

        if reps == 1:
            _emit_body(nc, tc, ctx, pools, dram)
        else:
            with tc.For_i(0, reps, 1):
                _emit_body(nc, tc, ctx, pools, dram)
        if timing_mode:
            tz = const.tile([1, 1], F32, tag="tz")
            nc.sync.dma_start(tz[:], dummy_in[:])
            nc.sync.dma_start(tiny_out[:], tz[:])

    nc.compile()
    return nc


_NC_CACHE = {}


def kernel(**inputs) -> np.ndarray:
    x = np.ascontiguousarray(np.asarray(inputs["x"], dtype=np.float32))
    w_qkv = np.ascontiguousarray(np.asarray(inputs["w_qkv"], dtype=np.float32))
    w_out = np.ascontiguousarray(np.asarray(inputs["w_out"], dtype=np.float32))
    b_out = np.ascontiguousarray(np.asarray(inputs["b_out"], dtype=np.float32))

    if "nc" not in _NC_CACHE:
        _NC_CACHE["nc"] = build_nc(reps=1)
    nc = _NC_CACHE["nc"]

    in_maps = [
        {"x": x[c], "w_qkv": w_qkv, "w_out": w_out, "b_out": b_out}
        for c in range(N_CORES)
    ]
    res = run_bass_kernel_spmd(nc, in_maps, core_ids=list(range(N_CORES)))
    out = np.stack([res.results[c]["out"] for c in range(N_CORES)], axis=0)
    return out.astype(np.float32)


# revision 39
# speedup vs baseline: 1.1162x; 1.1162x over previous
"""Trainium2 Bass kernel for nn_Attention (b=8, n=1024, dim=768, heads=12).

Sharding: data-parallel over batch — 8 batch elements -> 8 NeuronCores.
Each core runs full attention for one [1024, 768] slice; weights replicated.

Design (v1, software-pipelined):
  - exp is ACT-engine-only (no DVE/Pool exp on TRN2) and totals ~82us of the
    ~150us of PE matmul work, so attention is emitted as an ACT/PE pipeline
    and the q/k/v projections are interleaved INTO the attention rounds so PE
    fills its exp-wait slack with projection matmuls.
  - i-dim halved (512) in attention so PSUM fits: sts [128,2,512] (2 banks,
    x2 bufs) + AV accumulators [128,2,512] (2 banks) + proj fill (2 banks).
  - one exp instruction per round covers both heads of a pair (halves ACT
    instruction overhead vs per-head exps).
  - softmax denominators fold into the AV matmul via a ones-column on V;
    normalize = one fused reciprocal + one partition_broadcast + 2 muls,
    staged through SBUF so the PSUM accumulator frees fast.
  - out-projection k=0..4 accumulates into dead xT SBUF during the final
    (ACT-bound) segments; the tail is only k=5 + add + store, and the
    i-half-0 out tiles finish as units inside the last segment.
  - single ordered DMA queue, criticality-ordered (x strips / pair-0 qk
    weights / wv early; wo, bias late) — measured faster on HW than
    splitting across both HWDGE queues.
"""

import numpy as np
from contextlib import ExitStack

import concourse.bacc as bacc
import concourse.mybir as mybir
import concourse.tile as tile
from concourse.bass import ds, ts
from concourse.bass_utils import run_bass_kernel_spmd
from concourse.masks import make_identity

P = 128
N_CORES = 8
N_TOK = 1024
DIM = 768
H = 12
HD = 64
SCALE = 1.0 / (DIM ** 0.5)
F32 = mybir.dt.float32
F32R = mybir.dt.float32r
BF16 = mybir.dt.bfloat16
FP8 = mybir.dt.float8e4
DR = mybir.MatmulPerfMode.DoubleRow
QK_FP8 = False  # False: bf16 q/k stores, plain matmul (more accuracy margin)
AV_BF16 = False # bf16 vplus/es: halves AV matmul operand bandwidth
EXP = mybir.ActivationFunctionType.Exp

C_T = DIM // P          # 6  c-tiles
N_T = N_TOK // P        # 8  token tiles
IH = 512                # attention i-chunk (half of n per (pr, ihalf) pass)


def _emit_body(nc, tc, ctx, pools, dram):
    x_d, wqkv_d, wout_d, bout_d, out_d = dram
    const, persist, xpool, wpool, expool, npool, outpool, psS, psO, psP = pools

    wqkv_r = wqkv_d.rearrange("(o p) f -> p o f", p=P)
    wout_r = wout_d.rearrange("(o p) f -> p o f", p=P)

    # ---- constants ----
    identity = const.tile([P, P], F32, tag="ident")
    make_identity(nc, identity[:])

    # ---- persistent tensors ----
    # q/k stores are fp8e4 so QK^T runs in DoubleRow mode (0.5 cycles/row).
    # DoubleRow contracts two 64-row subtiles per instruction; the second
    # subtile of kT8 is zeroed, so its paired qT8 data is multiplied by 0.
    # qT8 carries a 512-col zeroed pad so the i-half-1 window's second
    # subtile reads in-bounds, finite data.
    xT = persist.tile([P, C_T, N_TOK], F32R, tag="xT")
    if QK_FP8:
        qT8 = persist.tile([P, 6, N_TOK + IH], FP8, tag="qT8")
        kT8 = persist.tile([P, 6, N_T, 2, P], FP8, tag="kT8")  # [pair, jt, sub, j]
    else:
        qkT = persist.tile([P, 6, 2, N_TOK], BF16, tag="qkT")  # [pair, q/k, n]
    vplus = persist.tile([P, N_T, H, HD + 1], BF16 if AV_BF16 else F32R,
                         tag="vplus")
    wv_sb = persist.tile([P, C_T, DIM], F32R, tag="wv")
    wo_sb = persist.tile([P, C_T, DIM], F32R, tag="wo")
    attnT = persist.tile([P, C_T, N_TOK], F32R, tag="attnT")

    if AV_BF16:
        nc.vector.memset(vplus[:, :, :, ds(HD, 1)], 1.0)
    else:
        nc.vector.memset(vplus[:, :, :, ds(HD, 1)].bitcast(F32), 1.0)
    # zero fills on the idle Pool engine: kT8's second subtiles, and qT8's
    # not-yet-written/pad columns that early rounds read as x0 garbage.
    # pair 0's slices go first so round (0,0,0) isn't gated on the bulk.
    if QK_FP8:
        nc.gpsimd.memset(kT8[:, 0, :, 1, :], 0.0)
        nc.gpsimd.memset(qT8[:, 0, ds(IH, N_TOK)], 0.0)
        nc.gpsimd.memset(kT8[:, ds(1, 5), :, 1, :], 0.0)
        nc.gpsimd.memset(qT8[:, ds(1, 5), ds(IH, N_TOK)], 0.0)

    # ---- DMA emission: one ordered queue, criticality order ----
    xs_tiles = []
    wqk_tiles = {}

    def load_wqk(fi):
        wt = wpool.tile([P, C_T, P], F32R, tag="wqk", name=f"wqk_{fi}")
        nc.sync.dma_start(wt[:], wqkv_r[:, :, ds(fi * P, P)].bitcast(F32R))
        wqk_tiles[fi] = wt

    for it in range(N_T):
        xs = xpool.tile([P, DIM], F32, tag="xs", name=f"xs_{it}")
        nc.sync.dma_start(xs[:], x_d[ts(it, P), :])
        xs_tiles.append(xs)
    load_wqk(0)
    load_wqk(6)
    nc.sync.dma_start(wv_sb[:], wqkv_r[:, :, ds(2 * DIM, DIM)].bitcast(F32R))
    load_wqk(1)
    load_wqk(7)
    b_row = const.tile([1, DIM], F32, tag="brow")
    nc.sync.dma_start(b_row[:], bout_d[None, :])
    bias_bc = const.tile([P, DIM], F32, tag="bias")
    nc.gpsimd.partition_broadcast(bias_bc[:], b_row[:])
    nc.sync.dma_start(wo_sb[:], wout_r[:].bitcast(F32R))

    # ---- x strip transpose (6 PE transposes + 3 merged DVE copies) ----
    def transpose_strip(it):
        xs = xs_tiles[it]
        for c in range(0, C_T, 2):
            pst = psS.tile([P, 2, IH], F32, tag="ps", name=f"t_{it}_{c}")
            nc.tensor.transpose(pst[:, 0, 0:P], xs[:, ts(c, P)], identity[:])
            nc.tensor.transpose(pst[:, 1, 0:P], xs[:, ts(c + 1, P)], identity[:])
            nc.vector.tensor_copy(xT[:, c:c + 2, ts(it, P)], pst[:, :, 0:P])

    # ---- projection units ----
    def qk_half(pair, qk, nh, pool):
        """half f-tile (fi = pair + 6*qk, n-half nh) -> qT8/kT8 (fp8 cast)."""
        fi = pair + 6 * qk
        wt = wqk_tiles[fi]
        ps = pool.tile([P, IH], F32, tag="ps" if pool is psS else "pp",
                       name=f"qk_{fi}_{nh}")
        for k in range(C_T):
            nc.tensor.matmul(
                ps[:], wt[:, k], xT[:, k, ds(nh * IH, IH)],
                start=(k == 0), stop=(k == C_T - 1))
        if not QK_FP8:
            nc.vector.tensor_copy(qkT[:, pair, qk, ds(nh * IH, IH)], ps[:])
        elif qk == 0:
            nc.vector.tensor_copy(qT8[:, pair, ds(nh * IH, IH)], ps[:])
        else:
            nc.vector.tensor_copy(
                kT8[:, pair, ds(nh * 4, 4), 0, :],
                ps[:].rearrange("p (j m) -> p j m", m=P))

    def v_half(jt, dh, pool):
        """V projection for d-chunk dh (0: d 0-511 / heads 0-7, 1: d 512-767
        / heads 8-11)."""
        off, w = (0, 512) if dh == 0 else (512, 256)
        ps = pool.tile([P, 512], F32, tag="ps" if pool is psS else "pp",
                       name=f"v_{jt}_{dh}")
        for k in range(C_T):
            nc.tensor.matmul(
                ps[:, ds(0, w)], xT[:, k, ts(jt, P)], wv_sb[:, k, ds(off, w)],
                start=(k == 0), stop=(k == C_T - 1))
        nc.vector.tensor_copy(
            vplus[:, jt, ds(off // HD, w // HD), ds(0, HD)],
            ps[:, :w].rearrange("p (h d) -> p h d", d=HD))

    # ---- pre-attention critical path: strips 0-3, pair-0 nh0 halves ----
    for it in range(4):
        transpose_strip(it)
    qk_half(0, 0, 0, psP)       # qT pair0, i-half 0
    qk_half(0, 1, 0, psS)       # kT pair0, j-tiles 0-3

    # ---- interleave schedule: everything else fills attention rounds ----
    # (pr, ihalf, jt) -> list of emit-callbacks run between QK(r+1) and AV(r).
    units = {}

    def at(pr, ihalf, jt, fn):
        units.setdefault((pr, ihalf, jt), []).append(fn)

    # segment 0: strips 4-7, V units just-in-time, pair-0 remaining halves
    at(0, 0, 0, lambda: transpose_strip(4))
    at(0, 0, 0, lambda: transpose_strip(5))
    at(0, 0, 0, lambda: v_half(0, 0, psP))
    at(0, 0, 0, lambda: v_half(0, 1, psP))
    at(0, 0, 1, lambda: transpose_strip(6))
    at(0, 0, 1, lambda: transpose_strip(7))
    at(0, 0, 1, lambda: v_half(1, 0, psP))
    at(0, 0, 1, lambda: v_half(1, 1, psP))
    at(0, 0, 2, lambda: qk_half(0, 1, 1, psP))      # kT pair0 j-tiles 4-7
    for j in range(2, N_T):     # V_j in round j, always ahead of AV(j)
        at(0, 0, j, lambda j=j: v_half(j, 0, psP))
        at(0, 0, j, lambda j=j: v_half(j, 1, psP))
    at(0, 0, 6, lambda: qk_half(0, 0, 1, psP))      # qT pair0, i-half 1
    # pair p>=1: all four halves inside segment 2p-1
    for p in range(1, 6):
        pr_, ih_ = divmod(2 * p - 1, 2)
        at(pr_, ih_, 0, lambda p=p: qk_half(p, 0, 0, psP))
        at(pr_, ih_, 2, lambda p=p: qk_half(p, 1, 0, psP))
        at(pr_, ih_, 4, lambda p=p: qk_half(p, 1, 1, psP))
        at(pr_, ih_, 6, lambda p=p: qk_half(p, 0, 1, psP))
        if p + 1 <= 5:          # prefetch pair p+1's weight tiles
            at(pr_, ih_, 0, lambda f=p + 1: load_wqk(f))
            at(pr_, ih_, 0, lambda f=p + 7: load_wqk(f))

    # out-projection k=0..4 partials run inside the (ACT-bound) final
    # segments; results stage into xT's SBUF, which is dead by then. Only
    # the k=5 matmul + add + store remain in the tail. Writes go through an
    # f32r-typed view (xT's location feeds f32r matmuls, and the verifier
    # requires f32r-rounded producers); tail reads use a plain-f32 bitcast.
    part_wr = xT.rearrange("p a b -> p (a b)")
    part_sb = xT.bitcast(F32).rearrange("p a b -> p (a b)")

    def part_unit(it):
        pp = psS.tile([P, 1024], F32, tag="ps", name=f"part_{it}")
        for k in range(5):
            for off, w in ((0, 512), (512, 256)):
                nc.tensor.matmul(
                    pp[:, ds(off, w)], attnT[:, k, ts(it, P)], wo_sb[:, k, ds(off, w)],
                    start=(k == 0), stop=(k == 4))
        nc.vector.tensor_add(part_wr[:, ds(it * DIM, DIM)], pp[:, :DIM],
                             bias_bc[:])

    for it in range(N_T):
        pr_, ih_ = divmod(10 + it // 4, 2)
        at(pr_, ih_, 2 * (it % 4), lambda it=it: part_unit(it))

    # k=5 + partial + store for the i-half-0 out tiles: their pair-5 rows
    # finish with segment 10, so they complete as units inside segment 11,
    # leaving only it4-7 after the last round.
    def k5_unit(it, pool_a, pool_b):
        ps_a = pool_a.tile([P, 512], F32, tag="ps" if pool_a is psS else "pp",
                           name=f"o5a_{it}")
        nc.tensor.matmul(ps_a[:, ds(0, 512)], attnT[:, 5, ts(it, P)],
                         wo_sb[:, 5, ds(0, 512)], start=True, stop=True)
        ps_b = pool_b.tile([P, 512], F32, tag="ps" if pool_b is psS else "pp",
                           name=f"o5b_{it}")
        nc.tensor.matmul(ps_b[:, ds(0, 256)], attnT[:, 5, ts(it, P)],
                         wo_sb[:, 5, ds(512, 256)], start=True, stop=True)
        if it < 4:
            os = outpool.tile([P, DIM], F32, tag="os", name=f"os_{it}")
        else:
            # x strip staging tiles are dead and exactly [128, DIM]: extra
            # store buffers so the last adds never wait on a completed DMA
            os = xpool.tile([P, DIM], F32, tag="xs", name=f"os_{it}")
        nc.vector.tensor_add(os[:, 0:512], ps_a[:, 0:512],
                             part_sb[:, ds(it * DIM, 512)])
        nc.vector.tensor_add(os[:, 512:DIM], ps_b[:, 0:256],
                             part_sb[:, ds(it * DIM + 512, 256)])
        nc.sync.dma_start(out_d[ts(it, P), :], os[:])

    for it in range(4):
        at(5, 1, 3 + it, lambda it=it: k5_unit(it, psP, psP))

    # ---- attention: flat pipeline over all (pr, ihalf, jt) rounds ----
    # Emission order per round r: exp(r), QK(r+1), proj-units, AV(r) — so the
    # PE's next QK is never queued behind an exp-gated AV, and ACT never
    # starves on the round chain.
    rounds = [(pr, ih, jt) for pr in range(6) for ih in range(2)
              for jt in range(N_T)]

    def emit_qk(pr, ihalf, jt):
        sts = psS.tile([P, 2, IH], F32, tag="ps", name=f"st_{pr}_{ihalf}_{jt}")
        for sub in range(2):
            b0 = HD * sub
            if QK_FP8:
                nc.tensor.matmul(
                    sts[:, sub, :],
                    kT8[b0:b0 + HD, pr, jt, :, :],
                    qT8[b0:b0 + HD, pr, ds(ihalf * IH, 2 * IH)].rearrange(
                        "p (s i) -> p s i", s=2),
                    start=True, stop=True, perf_mode=DR, tile_position=(b0, 0))
            else:
                nc.tensor.matmul(
                    sts[:, sub, :],
                    qkT[b0:b0 + HD, pr, 1, ts(jt, P)],
                    qkT[b0:b0 + HD, pr, 0, ds(ihalf * IH, IH)],
                    start=True, stop=True, tile_position=(b0, 0))
        return sts

    o_ps = None
    sts = emit_qk(*rounds[0])
    for i, (pr, ihalf, jt) in enumerate(rounds):
        es = expool.tile([P, 2, IH], BF16 if AV_BF16 else F32R, tag="es",
                         name=f"es_{pr}_{ihalf}_{jt}")
        nc.scalar.activation(es[:], sts[:], EXP, scale=SCALE)
        sts_next = emit_qk(*rounds[i + 1]) if i + 1 < len(rounds) else None
        for fn in units.get((pr, ihalf, jt), ()):
            fn()
        if jt == 0:
            o_ps = psO.tile([P, 2, IH], F32, tag="po", name=f"o_{pr}_{ihalf}")
        for sub in range(2):
            nc.tensor.matmul(
                o_ps[0:HD + 1, sub, :], vplus[:, jt, 2 * pr + sub],
                es[:, sub, :],
                start=(jt == 0), stop=(jt == N_T - 1))
        if jt == N_T - 1:
            last_seg = i == len(rounds) - 1
            if last_seg:
                # final segment: psO is never reused — normalize straight
                # from PSUM, off the staging-copy latency
                src = o_ps
            else:
                # one staging copy frees psO fast; normalize off SBUF after
                src = npool.tile([HD + 1, 2, IH], F32, tag="ostg", bufs=1,
                                 name=f"ostg_{pr}_{ihalf}")
                nc.vector.tensor_copy(src[:], o_ps[0:HD + 1, :, :])
            # fused normalize: one reciprocal + one broadcast cover both subs
            rec = npool.tile([1, 2, IH], F32, tag="rec")
            nc.vector.reciprocal(rec[:], src[HD:HD + 1, :, :])
            rb = npool.tile([HD, 2, IH], F32, tag="rb")
            nc.gpsimd.partition_broadcast(rb[:], rec[:])
            for sub in range(2):
                nc.vector.tensor_mul(
                    attnT[HD * sub:HD * (sub + 1), pr, ds(ihalf * IH, IH)],
                    src[0:HD, sub, :], rb[:, sub, :])
        sts = sts_next

    # ---- output projection tail: k=5 + staged partial for it4-7 ----
    for it in range(4, N_T):
        if it % 2 == 0:
            k5_unit(it, psS, psS)
        else:
            k5_unit(it, psP, psP)


def build_nc(reps: int = 1, timing_mode: bool = False):
    nc = bacc.Bacc("TRN2", target_bir_lowering=False, debug=False)
    if timing_mode:
        # device-resident garbage inputs: measure kernel exec, not host I/O
        x_d = nc.dram_tensor("x", [N_TOK, DIM], F32).ap()
        wqkv_d = nc.dram_tensor("w_qkv", [DIM, 3 * DIM], F32).ap()
        wout_d = nc.dram_tensor("w_out", [DIM, DIM], F32).ap()
        bout_d = nc.dram_tensor("b_out", [DIM], F32).ap()
        out_d = nc.dram_tensor("out", [N_TOK, DIM], F32).ap()
        dummy_in = nc.dram_tensor("dummy_in", [1, 1], F32, kind="ExternalInput").ap()
        tiny_out = nc.dram_tensor("tiny_out", [1, 1], F32, kind="ExternalOutput").ap()
    else:
        x_d = nc.dram_tensor("x", [N_TOK, DIM], F32, kind="ExternalInput").ap()
        wqkv_d = nc.dram_tensor("w_qkv", [DIM, 3 * DIM], F32, kind="ExternalInput").ap()
        wout_d = nc.dram_tensor("w_out", [DIM, DIM], F32, kind="ExternalInput").ap()
        bout_d = nc.dram_tensor("b_out", [DIM], F32, kind="ExternalInput").ap()
        out_d = nc.dram_tensor("out", [N_TOK, DIM], F32, kind="ExternalOutput").ap()
    dram = (x_d, wqkv_d, wout_d, bout_d, out_d)

    with ExitStack() as ctx:
        tc = ctx.enter_context(tile.TileContext(nc))
        const = ctx.enter_context(tc.tile_pool(name="const", bufs=1))
        persist = ctx.enter_context(tc.tile_pool(name="persist", bufs=1))
        xpool = ctx.enter_context(tc.tile_pool(name="xpool", bufs=3))
        wpool = ctx.enter_context(tc.tile_pool(name="wpool", bufs=3))
        expool = ctx.enter_context(tc.tile_pool(name="expool", bufs=4))
        npool = ctx.enter_context(tc.tile_pool(name="npool", bufs=2))
        outpool = ctx.enter_context(tc.tile_pool(name="outpool", bufs=2))
        psS = ctx.enter_context(tc.tile_pool(name="psS", bufs=2, space="PSUM"))
        psO = ctx.enter_context(tc.tile_pool(name="psO", bufs=1, space="PSUM"))
        psP = ctx.enter_context(tc.tile_pool(name="psP", bufs=2, space="PSUM"))
        pools = (const, persist, xpool, wpool, expool, npool, outpool, psS, psO, psP)

        if reps == 1:
            _emit_body(nc, tc, ctx, pools, dram)
        else:
            with tc.For_i(0, reps, 1):
                _emit_body(nc, tc, ctx, pools, dram)
        if timing_mode:
            tz = const.tile([1, 1], F32, tag="tz")
            nc.sync.dma_start(tz[:], dummy_in[:])
            nc.sync.dma_start(tiny_out[:], tz[:])

    nc.compile()
    return nc


_NC_CACHE = {}


def kernel(**inputs) -> np.ndarray:
    x = np.ascontiguousarray(np.asarray(inputs["x"], dtype=np.float32))
    w_qkv = np.ascontiguousarray(np.asarray(inputs["w_qkv"], dtype=np.float32))
    w_out = np.ascontiguousarray(np.asarray(inputs["w_out"], dtype=np.float32))
    b_out = np.ascontiguousarray(np.asarray(inputs["b_out"], dtype=np.float32))

    if "nc" not in _NC_CACHE:
        _NC_CACHE["nc"] = build_nc(reps=1)
    nc = _NC_CACHE["nc"]

    in_maps = [
        {"x": x[c], "w_qkv": w_qkv, "w_out": w_out, "b_out": b_out}
        for c in range(N_CORES)
    ]
    res = run_bass_kernel_spmd(nc, in_maps, core_ids=list(range(N_CORES)))
    out = np.stack([res.results[c]["out"] for c in range(N_CORES)], axis=0)
    return out.astype(np.float32)


# revision 41
# speedup vs baseline: 1.1168x; 1.0005x over previous
"""Trainium2 Bass kernel for nn_Attention (b=8, n=1024, dim=768, heads=12).

Sharding: data-parallel over batch — 8 batch elements -> 8 NeuronCores.
Each core runs full attention for one [1024, 768] slice; weights replicated.

Design (v1, software-pipelined):
  - exp is ACT-engine-only (no DVE/Pool exp on TRN2) and totals ~82us of the
    ~150us of PE matmul work, so attention is emitted as an ACT/PE pipeline
    and the q/k/v projections are interleaved INTO the attention rounds so PE
    fills its exp-wait slack with projection matmuls.
  - i-dim halved (512) in attention so PSUM fits: sts [128,2,512] (2 banks,
    x2 bufs) + AV accumulators [128,2,512] (2 banks) + proj fill (2 banks).
  - one exp instruction per round covers both heads of a pair (halves ACT
    instruction overhead vs per-head exps).
  - softmax denominators fold into the AV matmul via a ones-column on V;
    normalize = one fused reciprocal + one partition_broadcast + 2 muls,
    staged through SBUF so the PSUM accumulator frees fast.
  - out-projection k=0..4 accumulates into dead xT SBUF during the final
    (ACT-bound) segments; the tail is only k=5 + add + store, and the
    i-half-0 out tiles finish as units inside the last segment.
  - single ordered DMA queue, criticality-ordered (x strips / pair-0 qk
    weights / wv early; wo, bias late) — measured faster on HW than
    splitting across both HWDGE queues.
"""

import numpy as np
from contextlib import ExitStack

import concourse.bacc as bacc
import concourse.mybir as mybir
import concourse.tile as tile
from concourse.bass import ds, ts
from concourse.bass_utils import run_bass_kernel_spmd
from concourse.masks import make_identity

P = 128
N_CORES = 8
N_TOK = 1024
DIM = 768
H = 12
HD = 64
SCALE = 1.0 / (DIM ** 0.5)
F32 = mybir.dt.float32
F32R = mybir.dt.float32r
BF16 = mybir.dt.bfloat16
FP8 = mybir.dt.float8e4
DR = mybir.MatmulPerfMode.DoubleRow
QK_FP8 = False  # False: bf16 q/k stores, plain matmul (more accuracy margin)
AV_BF16 = False # bf16 vplus/es: halves AV matmul operand bandwidth
EXP = mybir.ActivationFunctionType.Exp

C_T = DIM // P          # 6  c-tiles
N_T = N_TOK // P        # 8  token tiles
IH = 512                # attention i-chunk (half of n per (pr, ihalf) pass)


def _emit_body(nc, tc, ctx, pools, dram):
    x_d, wqkv_d, wout_d, bout_d, out_d = dram
    const, persist, xpool, wpool, expool, npool, outpool, psS, psO, psP = pools

    wqkv_r = wqkv_d.rearrange("(o p) f -> p o f", p=P)
    wout_r = wout_d.rearrange("(o p) f -> p o f", p=P)

    # ---- constants ----
    identity = const.tile([P, P], F32, tag="ident")
    make_identity(nc, identity[:])

    # ---- persistent tensors ----
    # q/k stores are fp8e4 so QK^T runs in DoubleRow mode (0.5 cycles/row).
    # DoubleRow contracts two 64-row subtiles per instruction; the second
    # subtile of kT8 is zeroed, so its paired qT8 data is multiplied by 0.
    # qT8 carries a 512-col zeroed pad so the i-half-1 window's second
    # subtile reads in-bounds, finite data.
    xT = persist.tile([P, C_T, N_TOK], F32R, tag="xT")
    if QK_FP8:
        qT8 = persist.tile([P, 6, N_TOK + IH], FP8, tag="qT8")
        kT8 = persist.tile([P, 6, N_T, 2, P], FP8, tag="kT8")  # [pair, jt, sub, j]
    else:
        qkT = persist.tile([P, 6, 2, N_TOK], BF16, tag="qkT")  # [pair, q/k, n]
    vplus = persist.tile([P, N_T, H, HD + 1], BF16 if AV_BF16 else F32R,
                         tag="vplus")
    wv_sb = persist.tile([P, C_T, DIM], F32R, tag="wv")
    wo_sb = persist.tile([P, C_T, DIM], F32R, tag="wo")
    attnT = persist.tile([P, C_T, N_TOK], F32R, tag="attnT")

    if AV_BF16:
        nc.vector.memset(vplus[:, :, :, ds(HD, 1)], 1.0)
    else:
        nc.vector.memset(vplus[:, :, :, ds(HD, 1)].bitcast(F32), 1.0)
    # zero fills on the idle Pool engine: kT8's second subtiles, and qT8's
    # not-yet-written/pad columns that early rounds read as x0 garbage.
    # pair 0's slices go first so round (0,0,0) isn't gated on the bulk.
    if QK_FP8:
        nc.gpsimd.memset(kT8[:, 0, :, 1, :], 0.0)
        nc.gpsimd.memset(qT8[:, 0, ds(IH, N_TOK)], 0.0)
        nc.gpsimd.memset(kT8[:, ds(1, 5), :, 1, :], 0.0)
        nc.gpsimd.memset(qT8[:, ds(1, 5), ds(IH, N_TOK)], 0.0)

    # ---- DMA emission: one ordered queue, criticality order ----
    xs_tiles = []
    wqk_tiles = {}

    def load_wqk(fi):
        wt = wpool.tile([P, C_T, P], F32R, tag="wqk", name=f"wqk_{fi}")
        nc.sync.dma_start(wt[:], wqkv_r[:, :, ds(fi * P, P)].bitcast(F32R))
        wqk_tiles[fi] = wt

    for it in range(N_T):
        xs = xpool.tile([P, DIM], F32, tag="xs", name=f"xs_{it}")
        nc.sync.dma_start(xs[:], x_d[ts(it, P), :])
        xs_tiles.append(xs)
    load_wqk(0)
    load_wqk(6)
    nc.sync.dma_start(wv_sb[:], wqkv_r[:, :, ds(2 * DIM, DIM)].bitcast(F32R))
    load_wqk(1)
    load_wqk(7)
    b_row = const.tile([1, DIM], F32, tag="brow")
    nc.sync.dma_start(b_row[:], bout_d[None, :])
    bias_bc = const.tile([P, DIM], F32, tag="bias")
    nc.gpsimd.partition_broadcast(bias_bc[:], b_row[:])
    nc.sync.dma_start(wo_sb[:], wout_r[:].bitcast(F32R))

    # ---- x strip transpose (6 PE transposes + 3 merged DVE copies) ----
    def transpose_strip(it):
        xs = xs_tiles[it]
        for c in range(0, C_T, 2):
            pst = psS.tile([P, 2, IH], F32, tag="ps", name=f"t_{it}_{c}")
            nc.tensor.transpose(pst[:, 0, 0:P], xs[:, ts(c, P)], identity[:])
            nc.tensor.transpose(pst[:, 1, 0:P], xs[:, ts(c + 1, P)], identity[:])
            nc.vector.tensor_copy(xT[:, c:c + 2, ts(it, P)], pst[:, :, 0:P])

    # ---- projection units ----
    def qk_half(pair, qk, nh, pool):
        """half f-tile (fi = pair + 6*qk, n-half nh) -> qT8/kT8 (fp8 cast)."""
        fi = pair + 6 * qk
        wt = wqk_tiles[fi]
        ps = pool.tile([P, IH], F32, tag="ps" if pool is psS else "pp",
                       name=f"qk_{fi}_{nh}")
        for k in range(C_T):
            nc.tensor.matmul(
                ps[:], wt[:, k], xT[:, k, ds(nh * IH, IH)],
                start=(k == 0), stop=(k == C_T - 1))
        if not QK_FP8:
            nc.vector.tensor_copy(qkT[:, pair, qk, ds(nh * IH, IH)], ps[:])
        elif qk == 0:
            nc.vector.tensor_copy(qT8[:, pair, ds(nh * IH, IH)], ps[:])
        else:
            nc.vector.tensor_copy(
                kT8[:, pair, ds(nh * 4, 4), 0, :],
                ps[:].rearrange("p (j m) -> p j m", m=P))

    def v_half(jt, dh, pool):
        """V projection for d-chunk dh (0: d 0-511 / heads 0-7, 1: d 512-767
        / heads 8-11)."""
        off, w = (0, 512) if dh == 0 else (512, 256)
        ps = pool.tile([P, 512], F32, tag="ps" if pool is psS else "pp",
                       name=f"v_{jt}_{dh}")
        for k in range(C_T):
            nc.tensor.matmul(
                ps[:, ds(0, w)], xT[:, k, ts(jt, P)], wv_sb[:, k, ds(off, w)],
                start=(k == 0), stop=(k == C_T - 1))
        nc.vector.tensor_copy(
            vplus[:, jt, ds(off // HD, w // HD), ds(0, HD)],
            ps[:, :w].rearrange("p (h d) -> p h d", d=HD))

    # ---- pre-attention critical path: strips 0-3, pair-0 nh0 halves ----
    for it in range(4):
        transpose_strip(it)
    qk_half(0, 0, 0, psP)       # qT pair0, i-half 0
    qk_half(0, 1, 0, psS)       # kT pair0, j-tiles 0-3

    # ---- interleave schedule: everything else fills attention rounds ----
    # (pr, ihalf, jt) -> list of emit-callbacks run between QK(r+1) and AV(r).
    units = {}

    def at(pr, ihalf, jt, fn):
        units.setdefault((pr, ihalf, jt), []).append(fn)

    # segment 0: strips 4-7, V units just-in-time, pair-0 remaining halves
    at(0, 0, 0, lambda: transpose_strip(4))
    at(0, 0, 0, lambda: transpose_strip(5))
    at(0, 0, 0, lambda: v_half(0, 0, psP))
    at(0, 0, 0, lambda: v_half(0, 1, psP))
    at(0, 0, 1, lambda: transpose_strip(6))
    at(0, 0, 1, lambda: transpose_strip(7))
    at(0, 0, 1, lambda: v_half(1, 0, psP))
    at(0, 0, 1, lambda: v_half(1, 1, psP))
    at(0, 0, 2, lambda: qk_half(0, 1, 1, psP))      # kT pair0 j-tiles 4-7
    for j in range(2, N_T):     # V_j in round j, always ahead of AV(j)
        at(0, 0, j, lambda j=j: v_half(j, 0, psP))
        at(0, 0, j, lambda j=j: v_half(j, 1, psP))
    at(0, 0, 6, lambda: qk_half(0, 0, 1, psP))      # qT pair0, i-half 1
    # pair p>=1: all four halves inside segment 2p-1
    for p in range(1, 6):
        pr_, ih_ = divmod(2 * p - 1, 2)
        at(pr_, ih_, 0, lambda p=p: qk_half(p, 0, 0, psP))
        at(pr_, ih_, 2, lambda p=p: qk_half(p, 1, 0, psP))
        at(pr_, ih_, 4, lambda p=p: qk_half(p, 1, 1, psP))
        at(pr_, ih_, 6, lambda p=p: qk_half(p, 0, 1, psP))
        if p + 1 <= 5:          # prefetch pair p+1's weight tiles
            at(pr_, ih_, 0, lambda f=p + 1: load_wqk(f))
            at(pr_, ih_, 0, lambda f=p + 7: load_wqk(f))

    # out-projection k=0..4 partials run inside the (ACT-bound) final
    # segments; results stage into xT's SBUF, which is dead by then. Only
    # the k=5 matmul + add + store remain in the tail. Writes go through an
    # f32r-typed view (xT's location feeds f32r matmuls, and the verifier
    # requires f32r-rounded producers); tail reads use a plain-f32 bitcast.
    part_wr = xT.rearrange("p a b -> p (a b)")
    part_sb = xT.bitcast(F32).rearrange("p a b -> p (a b)")

    def part_unit(it):
        pp = psS.tile([P, 1024], F32, tag="ps", name=f"part_{it}")
        for k in range(5):
            for off, w in ((0, 512), (512, 256)):
                nc.tensor.matmul(
                    pp[:, ds(off, w)], attnT[:, k, ts(it, P)], wo_sb[:, k, ds(off, w)],
                    start=(k == 0), stop=(k == 4))
        nc.vector.tensor_add(part_wr[:, ds(it * DIM, DIM)], pp[:, :DIM],
                             bias_bc[:])

    for it in range(N_T):
        pr_, ih_ = divmod(10 + it // 4, 2)
        at(pr_, ih_, 2 * (it % 4), lambda it=it: part_unit(it))

    # k=5 + partial + store for the i-half-0 out tiles: their pair-5 rows
    # finish with segment 10, so they complete as units inside segment 11,
    # leaving only it4-7 after the last round.
    def k5_unit(it, pool_a, pool_b):
        ps_a = pool_a.tile([P, 512], F32, tag="ps" if pool_a is psS else "pp",
                           name=f"o5a_{it}")
        nc.tensor.matmul(ps_a[:, ds(0, 512)], attnT[:, 5, ts(it, P)],
                         wo_sb[:, 5, ds(0, 512)], start=True, stop=True)
        ps_b = pool_b.tile([P, 512], F32, tag="ps" if pool_b is psS else "pp",
                           name=f"o5b_{it}")
        nc.tensor.matmul(ps_b[:, ds(0, 256)], attnT[:, 5, ts(it, P)],
                         wo_sb[:, 5, ds(512, 256)], start=True, stop=True)
        if it < 4:
            os = outpool.tile([P, DIM], F32, tag="os", name=f"os_{it}")
        else:
            # x strip staging tiles are dead and exactly [128, DIM]: extra
            # store buffers so the last adds never wait on a completed DMA
            os = xpool.tile([P, DIM], F32, tag="xs", name=f"os_{it}")
        nc.vector.tensor_add(os[:, 0:512], ps_a[:, 0:512],
                             part_sb[:, ds(it * DIM, 512)])
        nc.vector.tensor_add(os[:, 512:DIM], ps_b[:, 0:256],
                             part_sb[:, ds(it * DIM + 512, 256)])
        nc.sync.dma_start(out_d[ts(it, P), :], os[:])

    for it in range(4):
        at(5, 1, 3 + it, lambda it=it: k5_unit(it, psP, psP))

    # ---- attention: flat pipeline over all (pr, ihalf, jt) rounds ----
    # Emission order per round r: exp(r), QK(r+1), proj-units, AV(r) — so the
    # PE's next QK is never queued behind an exp-gated AV, and ACT never
    # starves on the round chain.
    rounds = [(pr, ih, jt) for pr in range(6) for ih in range(2)
              for jt in range(N_T)]

    def emit_qk(pr, ihalf, jt):
        sts = psS.tile([P, 2, IH], F32, tag="ps", name=f"st_{pr}_{ihalf}_{jt}")
        for sub in range(2):
            b0 = HD * sub
            if QK_FP8:
                nc.tensor.matmul(
                    sts[:, sub, :],
                    kT8[b0:b0 + HD, pr, jt, :, :],
                    qT8[b0:b0 + HD, pr, ds(ihalf * IH, 2 * IH)].rearrange(
                        "p (s i) -> p s i", s=2),
                    start=True, stop=True, perf_mode=DR, tile_position=(b0, 0))
            else:
                nc.tensor.matmul(
                    sts[:, sub, :],
                    qkT[b0:b0 + HD, pr, 1, ts(jt, P)],
                    qkT[b0:b0 + HD, pr, 0, ds(ihalf * IH, IH)],
                    start=True, stop=True, tile_position=(b0, 0))
        return sts

    o_ps = None
    sts = emit_qk(*rounds[0])
    for i, (pr, ihalf, jt) in enumerate(rounds):
        es = expool.tile([P, 2, IH], BF16 if AV_BF16 else F32R, tag="es",
                         name=f"es_{pr}_{ihalf}_{jt}")
        nc.scalar.activation(es[:], sts[:], EXP, scale=SCALE)
        sts_next = emit_qk(*rounds[i + 1]) if i + 1 < len(rounds) else None
        for fn in units.get((pr, ihalf, jt), ()):
            fn()
        if jt == 0:
            o_ps = psO.tile([P, 2, IH], F32, tag="po", name=f"o_{pr}_{ihalf}")
        for sub in range(2):
            nc.tensor.matmul(
                o_ps[0:HD + 1, sub, :], vplus[:, jt, 2 * pr + sub],
                es[:, sub, :],
                start=(jt == 0), stop=(jt == N_T - 1))
        if jt == N_T - 1:
            last_seg = i == len(rounds) - 1
            if last_seg:
                # final segment: psO is never reused — normalize straight
                # from PSUM, off the staging-copy latency
                src = o_ps
            else:
                # one staging copy frees psO fast; normalize off SBUF after
                src = npool.tile([HD + 1, 2, IH], F32, tag="ostg", bufs=1,
                                 name=f"ostg_{pr}_{ihalf}")
                nc.vector.tensor_copy(src[:], o_ps[0:HD + 1, :, :])
            # fused normalize: one reciprocal + one broadcast cover both subs
            rec = npool.tile([1, 2, IH], F32, tag="rec")
            nc.vector.reciprocal(rec[:], src[HD:HD + 1, :, :])
            rb = npool.tile([HD, 2, IH], F32, tag="rb")
            nc.gpsimd.partition_broadcast(rb[:], rec[:])
            for sub in range(2):
                nc.vector.tensor_mul(
                    attnT[HD * sub:HD * (sub + 1), pr, ds(ihalf * IH, IH)],
                    src[0:HD, sub, :], rb[:, sub, :])
        sts = sts_next

    # ---- output projection tail: k=5 + staged partial for it4-7 ----
    for it in range(4, N_T):
        if it % 2 == 0:
            k5_unit(it, psS, psS)
        else:
            k5_unit(it, psP, psP)


def build_nc(reps: int = 1, timing_mode: bool = False):
    nc = bacc.Bacc("TRN2", target_bir_lowering=False, debug=False)
    if timing_mode:
        # device-resident garbage inputs: measure kernel exec, not host I/O
        x_d = nc.dram_tensor("x", [N_TOK, DIM], F32).ap()
        wqkv_d = nc.dram_tensor("w_qkv", [DIM, 3 * DIM], F32).ap()
        wout_d = nc.dram_tensor("w_out", [DIM, DIM], F32).ap()
        bout_d = nc.dram_tensor("b_out", [DIM], F32).ap()
        out_d = nc.dram_tensor("out", [N_TOK, DIM], F32).ap()
        dummy_in = nc.dram_tensor("dummy_in", [1, 1], F32, kind="ExternalInput").ap()
        tiny_out = nc.dram_tensor("tiny_out", [1, 1], F32, kind="ExternalOutput").ap()
    else:
        x_d = nc.dram_tensor("x", [N_TOK, DIM], F32, kind="ExternalInput").ap()
        wqkv_d = nc.dram_tensor("w_qkv", [DIM, 3 * DIM], F32, kind="ExternalInput").ap()
        wout_d = nc.dram_tensor("w_out", [DIM, DIM], F32, kind="ExternalInput").ap()
        bout_d = nc.dram_tensor("b_out", [DIM], F32, kind="ExternalInput").ap()
        out_d = nc.dram_tensor("out", [N_TOK, DIM], F32, kind="ExternalOutput").ap()
    dram = (x_d, wqkv_d, wout_d, bout_d, out_d)

    with ExitStack() as ctx:
        tc = ctx.enter_context(tile.TileContext(nc))
        const = ctx.enter_context(tc.tile_pool(name="const", bufs=1))
        persist = ctx.enter_context(tc.tile_pool(name="persist", bufs=1))
        xpool = ctx.enter_context(tc.tile_pool(name="xpool", bufs=3))
        wpool = ctx.enter_context(tc.tile_pool(name="wpool", bufs=3))
        expool = ctx.enter_context(tc.tile_pool(name="expool", bufs=4))
        npool = ctx.enter_context(tc.tile_pool(name="npool", bufs=2))
        outpool = ctx.enter_context(tc.tile_pool(name="outpool", bufs=2))
        psS = ctx.enter_context(tc.tile_pool(name="psS", bufs=2, space="PSUM"))
        psO = ctx.enter_context(tc.tile_pool(name="psO", bufs=1, space="PSUM"))
        psP = ctx.enter_context(tc.tile_pool(name="psP", bufs=2, space="PSUM"))
        pools = (const, persist, xpool, wpool, expool, npool, outpool, psS, psO, psP)

        if reps == 1:
            _emit_body(nc, tc, ctx, pools, dram)
        else:
            with tc.For_i(0, reps, 1):
                _emit_body(nc, tc, ctx, pools, dram)
        if timing_mode:
            tz = const.tile([1, 1], F32, tag="tz")
            nc.sync.dma_start(tz[:], dummy_in[:])
            nc.sync.dma_start(tiny_out[:], tz[:])

    nc.compile()
    return nc


_NC_CACHE = {}


def kernel(**inputs) -> np.ndarray:
    x = np.ascontiguousarray(np.asarray(inputs["x"], dtype=np.float32))
    w_qkv = np.ascontiguousarray(np.asarray(inputs["w_qkv"], dtype=np.float32))
    w_out = np.ascontiguousarray(np.asarray(inputs["w_out"], dtype=np.float32))
    b_out = np.ascontiguousarray(np.asarray(inputs["b_out"], dtype=np.float32))

    if "nc" not in _NC_CACHE:
        _NC_CACHE["nc"] = build_nc(reps=1)
    nc = _NC_CACHE["nc"]

    in_maps = [
        {"x": x[c], "w_qkv": w_qkv, "w_out": w_out, "b_out": b_out}
        for c in range(N_CORES)
    ]
    res = run_bass_kernel_spmd(nc, in_maps, core_ids=list(range(N_CORES)))
    out = np.stack([res.results[c]["out"] for c in range(N_CORES)], axis=0)
    return out.astype(np.float32)


# revision 50
# speedup vs baseline: 1.1342x; 1.0156x over previous
"""Trainium2 Bass kernel for nn_Attention (b=8, n=1024, dim=768, heads=12).

Sharding: data-parallel over batch — 8 batch elements -> 8 NeuronCores.
Each core runs full attention for one [1024, 768] slice; weights replicated.

Design (v1, software-pipelined):
  - exp is ACT-engine-only (no DVE/Pool exp on TRN2) and totals ~82us of the
    ~150us of PE matmul work, so attention is emitted as an ACT/PE pipeline
    and the q/k/v projections are interleaved INTO the attention rounds so PE
    fills its exp-wait slack with projection matmuls.
  - i-dim halved (512) in attention so PSUM fits: sts [128,2,512] (2 banks,
    x2 bufs) + AV accumulators [128,2,512] (2 banks) + proj fill (2 banks).
  - one exp instruction per round covers both heads of a pair (halves ACT
    instruction overhead vs per-head exps).
  - softmax denominators fold into the AV matmul via a ones-column on V;
    normalize = one fused reciprocal + one partition_broadcast + 2 muls,
    staged through SBUF so the PSUM accumulator frees fast.
  - out-projection k=0..4 accumulates into dead xT SBUF during the final
    (ACT-bound) segments; the final attention segment runs as two
    256-wide i-quarters so only 2 of 8 out tiles remain in the tail
    (k=5 matmul + add + store); the rest finish as in-segment units.
  - single ordered DMA queue, criticality-ordered (x strips / pair-0 qk
    weights / wv early; wo, bias late) — measured faster on HW than
    splitting across both HWDGE queues.
"""

import numpy as np
from contextlib import ExitStack

import concourse.bacc as bacc
import concourse.mybir as mybir
import concourse.tile as tile
from concourse.bass import ds, ts
from concourse.bass_utils import run_bass_kernel_spmd
from concourse.masks import make_identity

P = 128
N_CORES = 8
N_TOK = 1024
DIM = 768
H = 12
HD = 64
SCALE = 1.0 / (DIM ** 0.5)
F32 = mybir.dt.float32
F32R = mybir.dt.float32r
BF16 = mybir.dt.bfloat16
FP8 = mybir.dt.float8e4
DR = mybir.MatmulPerfMode.DoubleRow
QK_FP8 = False  # False: bf16 q/k stores, plain matmul (more accuracy margin)
AV_BF16 = False # bf16 vplus/es: halves AV matmul operand bandwidth
EXP = mybir.ActivationFunctionType.Exp

C_T = DIM // P          # 6  c-tiles
N_T = N_TOK // P        # 8  token tiles
IH = 512                # attention i-chunk (half of n per (pr, ihalf) pass)


def _emit_body(nc, tc, ctx, pools, dram):
    x_d, wqkv_d, wout_d, bout_d, out_d = dram
    const, persist, xpool, wpool, expool, npool, outpool, psS, psO, psP = pools

    wqkv_r = wqkv_d.rearrange("(o p) f -> p o f", p=P)
    wout_r = wout_d.rearrange("(o p) f -> p o f", p=P)

    # ---- constants ----
    identity = const.tile([P, P], F32, tag="ident")
    make_identity(nc, identity[:])

    # ---- persistent tensors ----
    # q/k stores are fp8e4 so QK^T runs in DoubleRow mode (0.5 cycles/row).
    # DoubleRow contracts two 64-row subtiles per instruction; the second
    # subtile of kT8 is zeroed, so its paired qT8 data is multiplied by 0.
    # qT8 carries a 512-col zeroed pad so the i-half-1 window's second
    # subtile reads in-bounds, finite data.
    xT = persist.tile([P, C_T, N_TOK], F32R, tag="xT")
    if QK_FP8:
        qT8 = persist.tile([P, 6, N_TOK + IH], FP8, tag="qT8")
        kT8 = persist.tile([P, 6, N_T, 2, P], FP8, tag="kT8")  # [pair, jt, sub, j]
    else:
        qkT = persist.tile([P, 6, 2, N_TOK], BF16, tag="qkT")  # [pair, q/k, n]
    vplus = persist.tile([P, N_T, H, HD + 1], BF16 if AV_BF16 else F32R,
                         tag="vplus")
    wv_sb = persist.tile([P, C_T, DIM], F32R, tag="wv")
    wo_sb = persist.tile([P, C_T, DIM], F32R, tag="wo")
    attnT = persist.tile([P, C_T, N_TOK], F32R, tag="attnT")

    if AV_BF16:
        nc.vector.memset(vplus[:, :, :, ds(HD, 1)], 1.0)
    else:
        nc.vector.memset(vplus[:, :, :, ds(HD, 1)].bitcast(F32), 1.0)
    # zero fills on the idle Pool engine: kT8's second subtiles, and qT8's
    # not-yet-written/pad columns that early rounds read as x0 garbage.
    # pair 0's slices go first so round (0,0,0) isn't gated on the bulk.
    if QK_FP8:
        nc.gpsimd.memset(kT8[:, 0, :, 1, :], 0.0)
        nc.gpsimd.memset(qT8[:, 0, ds(IH, N_TOK)], 0.0)
        nc.gpsimd.memset(kT8[:, ds(1, 5), :, 1, :], 0.0)
        nc.gpsimd.memset(qT8[:, ds(1, 5), ds(IH, N_TOK)], 0.0)

    # ---- DMA emission: one ordered queue, criticality order ----
    xs_tiles = []
    wqk_tiles = {}

    def load_wqk(fi):
        wt = wpool.tile([P, C_T, P], F32R, tag="wqk", name=f"wqk_{fi}")
        nc.sync.dma_start(wt[:], wqkv_r[:, :, ds(fi * P, P)].bitcast(F32R))
        wqk_tiles[fi] = wt

    for it in range(N_T):
        xs = xpool.tile([P, DIM], F32, tag="xs", name=f"xs_{it}")
        if it < 4:
            nc.sync.dma_start(xs[:], x_d[ts(it, P), :])
        xs_tiles.append(xs)
    load_wqk(0)
    load_wqk(6)
    nc.sync.dma_start(wv_sb[:], wqkv_r[:, :, ds(2 * DIM, DIM)].bitcast(F32R))
    for it in range(4, N_T):
        nc.sync.dma_start(xs_tiles[it][:], x_d[ts(it, P), :])
    load_wqk(1)
    load_wqk(7)
    b_row = const.tile([1, DIM], F32, tag="brow")
    nc.sync.dma_start(b_row[:], bout_d[None, :])
    bias_bc = const.tile([P, DIM], F32, tag="bias")
    nc.gpsimd.partition_broadcast(bias_bc[:], b_row[:])
    nc.sync.dma_start(wo_sb[:], wout_r[:].bitcast(F32R))

    # ---- x strip transpose (6 PE transposes + 3 merged DVE copies) ----
    def transpose_strip(it):
        xs = xs_tiles[it]
        for c in range(0, C_T, 2):
            pst = psS.tile([P, 2, IH], F32, tag="ps", name=f"t_{it}_{c}")
            nc.tensor.transpose(pst[:, 0, 0:P], xs[:, ts(c, P)], identity[:])
            nc.tensor.transpose(pst[:, 1, 0:P], xs[:, ts(c + 1, P)], identity[:])
            nc.vector.tensor_copy(xT[:, c:c + 2, ts(it, P)], pst[:, :, 0:P])

    # ---- projection units ----
    def qk_half(pair, qk, nh, pool):
        """half f-tile (fi = pair + 6*qk, n-half nh) -> qT8/kT8 (fp8 cast)."""
        fi = pair + 6 * qk
        wt = wqk_tiles[fi]
        ps = pool.tile([P, IH], F32, tag="ps" if pool is psS else "pp",
                       name=f"qk_{fi}_{nh}")
        for k in range(C_T):
            nc.tensor.matmul(
                ps[:], wt[:, k], xT[:, k, ds(nh * IH, IH)],
                start=(k == 0), stop=(k == C_T - 1))
        if not QK_FP8:
            nc.vector.tensor_copy(qkT[:, pair, qk, ds(nh * IH, IH)], ps[:])
        elif qk == 0:
            nc.vector.tensor_copy(qT8[:, pair, ds(nh * IH, IH)], ps[:])
        else:
            nc.vector.tensor_copy(
                kT8[:, pair, ds(nh * 4, 4), 0, :],
                ps[:].rearrange("p (j m) -> p j m", m=P))

    def v_half(jt, dh, pool):
        """V projection for d-chunk dh (0: d 0-511 / heads 0-7, 1: d 512-767
        / heads 8-11)."""
        off, w = (0, 512) if dh == 0 else (512, 256)
        ps = pool.tile([P, 512], F32, tag="ps" if pool is psS else "pp",
                       name=f"v_{jt}_{dh}")
        for k in range(C_T):
            nc.tensor.matmul(
                ps[:, ds(0, w)], xT[:, k, ts(jt, P)], wv_sb[:, k, ds(off, w)],
                start=(k == 0), stop=(k == C_T - 1))
        nc.vector.tensor_copy(
            vplus[:, jt, ds(off // HD, w // HD), ds(0, HD)],
            ps[:, :w].rearrange("p (h d) -> p h d", d=HD))

    # ---- pre-attention critical path: strips 0-3, pair-0 nh0 halves ----
    for it in range(4):
        transpose_strip(it)
    qk_half(0, 0, 0, psP)       # qT pair0, i-half 0
    qk_half(0, 1, 0, psS)       # kT pair0, j-tiles 0-3

    # ---- interleave schedule: everything else fills attention rounds ----
    # (pr, ihalf, jt) -> list of emit-callbacks run between QK(r+1) and AV(r).
    units = {}

    def at(pr, ihalf, jt, fn):
        units.setdefault((pr, ihalf, jt), []).append(fn)

    # segment 0: strips 4-7, V units just-in-time, pair-0 remaining halves
    at(0, 0, 0, lambda: transpose_strip(4))
    at(0, 0, 0, lambda: transpose_strip(5))
    at(0, 0, 0, lambda: v_half(0, 0, psP))
    at(0, 0, 0, lambda: v_half(0, 1, psP))
    at(0, 0, 1, lambda: transpose_strip(6))
    at(0, 0, 1, lambda: transpose_strip(7))
    at(0, 0, 1, lambda: v_half(1, 0, psP))
    at(0, 0, 1, lambda: v_half(1, 1, psP))
    at(0, 0, 2, lambda: qk_half(0, 1, 1, psP))      # kT pair0 j-tiles 4-7
    for j in range(2, N_T):     # V_j in round j, always ahead of AV(j)
        at(0, 0, j, lambda j=j: v_half(j, 0, psP))
        at(0, 0, j, lambda j=j: v_half(j, 1, psP))
    at(0, 0, 6, lambda: qk_half(0, 0, 1, psP))      # qT pair0, i-half 1
    # pair p>=1: all four halves inside segment 2p-1
    for p in range(1, 6):
        pr_, ih_ = divmod(2 * p - 1, 2)
        at(pr_, ih_, 0, lambda p=p: qk_half(p, 0, 0, psP))
        at(pr_, ih_, 2, lambda p=p: qk_half(p, 1, 0, psP))
        at(pr_, ih_, 4, lambda p=p: qk_half(p, 1, 1, psP))
        at(pr_, ih_, 6, lambda p=p: qk_half(p, 0, 1, psP))
        if p + 1 <= 5:          # prefetch pair p+1's weight tiles
            at(pr_, ih_, 0, lambda f=p + 1: load_wqk(f))
            at(pr_, ih_, 0, lambda f=p + 7: load_wqk(f))

    # out-projection k=0..4 partials run inside the (ACT-bound) final
    # segments; results stage into xT's SBUF, which is dead by then. Only
    # the k=5 matmul + add + store remain in the tail. Writes go through an
    # f32r-typed view (xT's location feeds f32r matmuls, and the verifier
    # requires f32r-rounded producers); tail reads use a plain-f32 bitcast.
    part_wr = xT.rearrange("p a b -> p (a b)")
    part_sb = xT.bitcast(F32).rearrange("p a b -> p (a b)")

    def part_unit(it):
        pp = psS.tile([P, 1024], F32, tag="ps", name=f"part_{it}")
        for k in range(5):
            for off, w in ((0, 512), (512, 256)):
                nc.tensor.matmul(
                    pp[:, ds(off, w)], attnT[:, k, ts(it, P)], wo_sb[:, k, ds(off, w)],
                    start=(k == 0), stop=(k == 4))
        nc.vector.tensor_add(part_wr[:, ds(it * DIM, DIM)], pp[:, :DIM],
                             bias_bc[:])

    for it in range(N_T):
        pr_, ih_ = divmod(10 + it // 4, 2)
        at(pr_, ih_, 2 * (it % 4), lambda it=it: part_unit(it))

    # k=5 + partial + store for the i-half-0 out tiles: their pair-5 rows
    # finish with segment 10, so they complete as units inside segment 11,
    # leaving only it4-7 after the last round.
    def k5_unit(it, pool_a, pool_b):
        ps_a = pool_a.tile([P, 512], F32, tag="ps" if pool_a is psS else "pp",
                           name=f"o5a_{it}")
        nc.tensor.matmul(ps_a[:, ds(0, 512)], attnT[:, 5, ts(it, P)],
                         wo_sb[:, 5, ds(0, 512)], start=True, stop=True)
        ps_b = pool_b.tile([P, 512], F32, tag="ps" if pool_b is psS else "pp",
                           name=f"o5b_{it}")
        nc.tensor.matmul(ps_b[:, ds(0, 256)], attnT[:, 5, ts(it, P)],
                         wo_sb[:, 5, ds(512, 256)], start=True, stop=True)
        if it < 4:
            os = outpool.tile([P, DIM], F32, tag="os", name=f"os_{it}")
        else:
            # x strip staging tiles are dead and exactly [128, DIM]: extra
            # store buffers so the last adds never wait on a completed DMA
            os = xpool.tile([P, DIM], F32, tag="xs", name=f"os_{it}")
        nc.vector.tensor_add(os[:, 0:512], ps_a[:, 0:512],
                             part_sb[:, ds(it * DIM, 512)])
        nc.vector.tensor_add(os[:, 512:DIM], ps_b[:, 0:256],
                             part_sb[:, ds(it * DIM + 512, 256)])
        nc.sync.dma_start(out_d[ts(it, P), :], os[:])

    for it in range(4):
        at(5, 1, 3 + it, lambda it=it: k5_unit(it, psP, psP))

    # ---- attention: flat pipeline over all rounds ----
    # Emission order per round r: exp(r), QK(r+1), proj-units, AV(r) — so the
    # PE's next QK is never queued behind an exp-gated AV, and ACT never
    # starves on the round chain. The final segment (pr5, i-half 1) runs as
    # two 256-wide i-quarters so the end-of-kernel chain (normalize -> k=5 ->
    # store) covers 2 output tiles instead of 4; it4/it5 finish as units
    # inside the last quarter.
    rounds = []   # (pr, ibase, iw, jt, ukey)
    for pr in range(6):
        for ih in range(2):
            if pr == 5 and ih == 1:
                for qb, qbase in enumerate((IH, IH + 256)):
                    for jt in range(N_T):
                        rounds.append((pr, qbase, 256, jt, (5, 1 + qb, jt)))
            else:
                for jt in range(N_T):
                    rounds.append((pr, ih * IH, IH, jt, (pr, ih, jt)))
    at(5, 2, 3, lambda: k5_unit(4, psP, psP))
    at(5, 2, 5, lambda: k5_unit(5, psP, psP))

    def emit_qk(pr, ibase, iw, jt, ukey):
        # tiles stay full-width so each sub's accumulation group owns its
        # own PSUM bank even when iw < IH; only [:, sub, 0:iw] is used
        sts = psS.tile([P, 2, IH], F32, tag="ps", name=f"st_{ukey}")
        sts = sts[:, :, 0:iw]
        for sub in range(2):
            b0 = HD * sub
            if QK_FP8:
                nc.tensor.matmul(
                    sts[:, sub, :],
                    kT8[b0:b0 + HD, pr, jt, :, :],
                    qT8[b0:b0 + HD, pr, ds(ibase, 2 * iw)].rearrange(
                        "p (s i) -> p s i", s=2),
                    start=True, stop=True, perf_mode=DR, tile_position=(b0, 0))
            else:
                nc.tensor.matmul(
                    sts[:, sub, :],
                    qkT[b0:b0 + HD, pr, 1, ts(jt, P)],
                    qkT[b0:b0 + HD, pr, 0, ds(ibase, iw)],
                    start=True, stop=True, tile_position=(b0, 0))
        return sts

    o_ps = None
    sts = emit_qk(*rounds[0])
    for i, (pr, ibase, iw, jt, ukey) in enumerate(rounds):
        es = expool.tile([P, 2, iw], BF16 if AV_BF16 else F32R, tag="es",
                         name=f"es_{ukey}")
        nc.scalar.activation(es[:], sts[:], EXP, scale=SCALE)
        sts_next = emit_qk(*rounds[i + 1]) if i + 1 < len(rounds) else None
        for fn in units.get(ukey, ()):
            fn()
        if jt == 0:
            o_ps = psO.tile([P, 2, IH], F32, tag="po", name=f"o_{ukey}")
            o_ps = o_ps[:, :, 0:iw]
        for sub in range(2):
            nc.tensor.matmul(
                o_ps[0:HD + 1, sub, :], vplus[:, jt, 2 * pr + sub],
                es[:, sub, :],
                start=(jt == 0), stop=(jt == N_T - 1))
        if jt == N_T - 1:
            last_seg = i == len(rounds) - 1
            if last_seg:
                # final segment: psO is never reused — normalize straight
                # from PSUM, off the staging-copy latency
                src = o_ps
            else:
                # one staging copy frees psO fast; normalize off SBUF after
                src = npool.tile([HD + 1, 2, iw], F32, tag="ostg", bufs=1,
                                 name=f"ostg_{ukey}")
                nc.vector.tensor_copy(src[:], o_ps[0:HD + 1, :, :])
            # fused normalize: one reciprocal + one broadcast cover both subs
            rec = npool.tile([1, 2, iw], F32, tag="rec")
            nc.vector.reciprocal(rec[:], src[HD:HD + 1, :, :])
            rb = npool.tile([HD, 2, iw], F32, tag="rb")
            nc.gpsimd.partition_broadcast(rb[:], rec[:])
            for sub in range(2):
                nc.vector.tensor_mul(
                    attnT[HD * sub:HD * (sub + 1), pr, ds(ibase, iw)],
                    src[0:HD, sub, :], rb[:, sub, :])
        sts = sts_next

    # ---- output projection tail: k=5 + staged partial for it6-7 ----
    k5_unit(6, psS, psS)
    k5_unit(7, psP, psP)


def build_nc(reps: int = 1, timing_mode: bool = False):
    nc = bacc.Bacc("TRN2", target_bir_lowering=False, debug=False)
    if timing_mode:
        # device-resident garbage inputs: measure kernel exec, not host I/O
        x_d = nc.dram_tensor("x", [N_TOK, DIM], F32).ap()
        wqkv_d = nc.dram_tensor("w_qkv", [DIM, 3 * DIM], F32).ap()
        wout_d = nc.dram_tensor("w_out", [DIM, DIM], F32).ap()
        bout_d = nc.dram_tensor("b_out", [DIM], F32).ap()
        out_d = nc.dram_tensor("out", [N_TOK, DIM], F32).ap()
        dummy_in = nc.dram_tensor("dummy_in", [1, 1], F32, kind="ExternalInput").ap()
        tiny_out = nc.dram_tensor("tiny_out", [1, 1], F32, kind="ExternalOutput").ap()
    else:
        x_d = nc.dram_tensor("x", [N_TOK, DIM], F32, kind="ExternalInput").ap()
        wqkv_d = nc.dram_tensor("w_qkv", [DIM, 3 * DIM], F32, kind="ExternalInput").ap()
        wout_d = nc.dram_tensor("w_out", [DIM, DIM], F32, kind="ExternalInput").ap()
        bout_d = nc.dram_tensor("b_out", [DIM], F32, kind="ExternalInput").ap()
        out_d = nc.dram_tensor("out", [N_TOK, DIM], F32, kind="ExternalOutput").ap()
    dram = (x_d, wqkv_d, wout_d, bout_d, out_d)

    with ExitStack() as ctx:
        tc = ctx.enter_context(tile.TileContext(nc))
        const = ctx.enter_context(tc.tile_pool(name="const", bufs=1))
        persist = ctx.enter_context(tc.tile_pool(name="persist", bufs=1))
        xpool = ctx.enter_context(tc.tile_pool(name="xpool", bufs=3))
        wpool = ctx.enter_context(tc.tile_pool(name="wpool", bufs=3))
        expool = ctx.enter_context(tc.tile_pool(name="expool", bufs=4))
        npool = ctx.enter_context(tc.tile_pool(name="npool", bufs=2))
        outpool = ctx.enter_context(tc.tile_pool(name="outpool", bufs=2))
        psS = ctx.enter_context(tc.tile_pool(name="psS", bufs=2, space="PSUM"))
        psO = ctx.enter_context(tc.tile_pool(name="psO", bufs=1, space="PSUM"))
        psP = ctx.enter_context(tc.tile_pool(name="psP", bufs=2, space="PSUM"))
        pools = (const, persist, xpool, wpool, expool, npool, outpool, psS, psO, psP)

        if reps == 1:
            _emit_body(nc, tc, ctx, pools, dram)
        else:
            with tc.For_i(0, reps, 1):
                _emit_body(nc, tc, ctx, pools, dram)
        if timing_mode:
            tz = const.tile([1, 1], F32, tag="tz")
            nc.sync.dma_start(tz[:], dummy_in[:])
            nc.sync.dma_start(tiny_out[:], tz[:])

    nc.compile()
    return nc


_NC_CACHE = {}


def kernel(**inputs) -> np.ndarray:
    x = np.ascontiguousarray(np.asarray(inputs["x"], dtype=np.float32))
    w_qkv = np.ascontiguousarray(np.asarray(inputs["w_qkv"], dtype=np.float32))
    w_out = np.ascontiguousarray(np.asarray(inputs["w_out"], dtype=np.float32))
    b_out = np.ascontiguousarray(np.asarray(inputs["b_out"], dtype=np.float32))

    if "nc" not in _NC_CACHE:
        _NC_CACHE["nc"] = build_nc(reps=1)
    nc = _NC_CACHE["nc"]

    in_maps = [
        {"x": x[c], "w_qkv": w_qkv, "w_out": w_out, "b_out": b_out}
        for c in range(N_CORES)
    ]
    res = run_bass_kernel_spmd(nc, in_maps, core_ids=list(range(N_CORES)))
    out = np.stack([res.results[c]["out"] for c in range(N_CORES)], axis=0)
    return out.astype(np.float32)
